# revision 4
# baseline (speedup 1.0000x reference)
"""Trainium2 Bass kernel for LeViT-style attention (nn_Attention_21981642621177).

y = proj(softmax(q k^T * scale + ab) v) with
B=2048, N=49 tokens, DIM=384, HEADS=8, KEY_DIM=32, D=128 (per-head v), DH=1024.

Sharding: pure data parallel over batch across 8 NeuronCores (256 batches/core).

Device program (per core) is the same layout strategy as the original:
  - x is loaded token-major [128 tok, 384] directly as bf16, PE-transposed to
    feature-major xT [384, tokens].
  - qT/kT = W^T-stationary matmuls on xT -> feature-major [256, tokens] bf16.
  - scores per (2-batch group, head) via tile-position-packed matmuls; softmax
    via exp + precomputed exp(bias) table + mask-matmul denominators.
  - out per (batch, head) with v stationary; y = proj, written out as bf16.

Host/exec path: the run_bass_kernel_spmd/axon path rebuilds the jitted
function and re-ships ~460MB (f32 x + zero output donation buffers + f32 y +
consts) over the axon tunnel on EVERY call. That dominated wall time. This
version instead:
  - builds jit(shard_map(bass_exec)) once and caches it,
  - keeps the (tiny) weight/bias consts device-resident across calls,
  - ships x as bf16 (77MB) with per-shard host cast + transfer in threads,
  - returns y as bf16 and casts to f32 on the host in threads,
  - donates the previous call's y device buffer as the next call's output
    scratch (the kernel writes every element of y, so no zeroing is needed).
"""

import numpy as np
import ml_dtypes
from concurrent.futures import ThreadPoolExecutor
from contextlib import ExitStack

import concourse.bass as bass
import concourse.bacc as bacc_mod
import concourse.tile as tile
from concourse import mybir

BF16 = mybir.dt.bfloat16
F32 = mybir.dt.float32
NPBF = ml_dtypes.bfloat16

B, N, DIM, HEADS, KD, D, DH, NHKD = 2048, 49, 384, 8, 32, 128, 1024, 256
SCALE = KD ** -0.5
NCORES = 8
BPC = B // NCORES              # 256 batches per core
T = BPC * N                    # 12544 tokens per core
HALF_B = 128                   # batches per half
HALF_T = HALF_B * N            # 6272 tokens per half (= 49 tiles of 128)
CHUNK = 896                    # 7 tiles of 128 tokens
NCHUNK = HALF_T // CHUNK       # 7
QKW = 960                      # q/k chunk width (64-col overlap for ragged reads)
GW = HEADS * N                 # 392 free width of score/probs banks


def _build():
    nc = bacc_mod.Bacc(None)
    x_d = nc.dram_tensor("x", [T, DIM], BF16, kind="ExternalInput")
    qw_d = nc.dram_tensor("qw", [3, 128, NHKD], BF16, kind="ExternalInput")
    kw_d = nc.dram_tensor("kw", [3, 128, NHKD], BF16, kind="ExternalInput")
    vw_d = nc.dram_tensor("vw", [3, 128, DH], BF16, kind="ExternalInput")
    pw_d = nc.dram_tensor("pw", [HEADS, 128, DIM], BF16, kind="ExternalInput")
    eab_d = nc.dram_tensor("eab", [128, GW], BF16, kind="ExternalInput")
    mask_d = nc.dram_tensor("mask", [16, 128, 32], BF16, kind="ExternalInput")
    id_d = nc.dram_tensor("ident", [128, 128], BF16, kind="ExternalInput")
    y_d = nc.dram_tensor("y", [T, DIM], BF16, kind="ExternalOutput")

    with tile.TileContext(nc) as tc, ExitStack() as ctx:
        consts = ctx.enter_context(tc.tile_pool(name="consts", bufs=1))
        xbf_p = ctx.enter_context(tc.tile_pool(name="xbf", bufs=3))
        xT_p = ctx.enter_context(tc.tile_pool(name="xT", bufs=1))
        qk_p = ctx.enter_context(tc.tile_pool(name="qk", bufs=3))
        pr_p = ctx.enter_context(tc.tile_pool(name="probs", bufs=1))
        v_p = ctx.enter_context(tc.tile_pool(name="vch", bufs=2))
        v2_p = ctx.enter_context(tc.tile_pool(name="v2", bufs=4))
        hT_p = ctx.enter_context(tc.tile_pool(name="hT", bufs=2))
        rc_p = ctx.enter_context(tc.tile_pool(name="recip", bufs=1))
        bc_p = ctx.enter_context(tc.tile_pool(name="bcast", bufs=2))
        y_p = ctx.enter_context(tc.tile_pool(name="yout", bufs=2))
        dr_p = ctx.enter_context(tc.tile_pool(name="dram", bufs=2, space="DRAM"))
        sh_ps = ctx.enter_context(tc.tile_pool(name="shps", bufs=2, space="PSUM"))
        sc_ps = ctx.enter_context(tc.tile_pool(name="scps", bufs=1, space="PSUM"))
        su_ps = ctx.enter_context(tc.tile_pool(name="sups", bufs=1, space="PSUM"))
        ht_ps = ctx.enter_context(tc.tile_pool(name="htps", bufs=1, space="PSUM"))

        qw_sb = consts.tile([128, 3, NHKD], BF16, tag="qw")
        nc.sync.dma_start(qw_sb[:], qw_d.rearrange("a p b -> p a b"))
        kw_sb = consts.tile([128, 3, NHKD], BF16, tag="kw")
        nc.sync.dma_start(kw_sb[:], kw_d.rearrange("a p b -> p a b"))
        vw_sb = consts.tile([128, 3, DH], BF16, tag="vw")
        nc.sync.dma_start(vw_sb[:], vw_d.rearrange("a p b -> p a b"))
        pw_sb = consts.tile([128, HEADS, DIM], BF16, tag="pw")
        nc.sync.dma_start(pw_sb[:], pw_d.rearrange("a p b -> p a b"))
        eab_sb = consts.tile([128, GW], BF16, tag="eab")
        nc.sync.dma_start(eab_sb[:], eab_d[:])
        mask_sb = consts.tile([128, 16, 32], BF16, tag="mask")
        nc.sync.dma_start(mask_sb[:], mask_d.rearrange("a p b -> p a b"))
        id_sb = consts.tile([128, 128], BF16, tag="ident")
        nc.sync.dma_start(id_sb[:], id_d[:])

        for half in range(2):
            ht0 = half * HALF_T  # global token offset of this half

            xT = xT_p.tile([128, 3, HALF_T], BF16, tag="xT")
            probs = pr_p.tile([128, 64, GW], BF16, tag="probs")
            sums_b = su_ps.tile([128, 512], F32, tag="sums")
            sums = sums_b[:, :GW]

            # ---- P1+P2: transpose x, project q/k (per chunk) ----
            # qk for chunk c reads a 64-col overlap into chunk c+1, so it is
            # emitted only after chunk c+1's transposes exist.
            qk_tiles = []

            def emit_qk(c):
                c0 = c * CHUNK
                qc = qk_p.tile([128, 2, QKW], BF16, tag="qc")
                kc = qk_p.tile([128, 2, QKW], BF16, tag="kc")
                # columns beyond the half's tokens don't exist: zero the tail
                w_av = min(QKW, HALF_T - c0)  # 960, or 896 on last chunk
                if w_av < QKW:
                    nc.vector.memset(qc[:], 0.0)
                    nc.vector.memset(kc[:], 0.0)
                for wsb, dst in ((qw_sb, qc), (kw_sb, kc)):
                    for m in range(2):
                        for o0, w in ((0, 512), (512, w_av - 512)):
                            ps = sh_ps.tile([128, 512], F32, tag="shps")
                            for f in range(3):
                                nc.tensor.matmul(
                                    ps[:, :w],
                                    wsb[:, f, 128 * m : 128 * (m + 1)],
                                    xT[:, f, c0 + o0 : c0 + o0 + w],
                                    start=(f == 0),
                                    stop=(f == 2),
                                )
                            nc.scalar.copy(dst[:, m, o0 : o0 + w], ps[:, :w])
                qk_tiles.append((qc, kc))

            for c in range(NCHUNK):
                c0 = c * CHUNK
                for t in range(7):
                    t0 = c0 + t * 128
                    xb = xbf_p.tile([128, DIM], BF16, tag="xbf")
                    nc.gpsimd.dma_start(xb[:], x_d[ht0 + t0 : ht0 + t0 + 128, :])
                    for j in range(3):
                        pt = sh_ps.tile([128, 128], BF16, tag="shps")
                        nc.tensor.transpose(pt[:], xb[:, 128 * j : 128 * (j + 1)], id_sb[:])
                        nc.vector.tensor_copy(xT[:, j, t0 : t0 + 128], pt[:])
                if c > 0:
                    emit_qk(c - 1)
            emit_qk(NCHUNK - 1)

            # ---- P3: scores + exp + bias-mul + denominator sums (per group) ----
            for g in range(64):
                # one bank per lhsT row group: concurrent row-packed matmuls
                # into a single bank collide on hardware
                sc4 = sc_ps.tile([128, 4, 512], F32, tag="scps")
                for h in range(HEADS):
                    s, r0 = h // 4, 32 * (h % 4)
                    for sub in range(2):
                        b = 2 * g + sub
                        col = N * b
                        cq, oq = col // CHUNK, col % CHUNK
                        qc, kc = qk_tiles[cq]
                        nc.tensor.matmul(
                            sc4[64 * sub : 64 * sub + 64, h % 4, N * s : N * s + N],
                            kc[r0 : r0 + 32, s, oq : oq + 64],
                            qc[r0 : r0 + 32, s, oq : oq + N],
                            start=True,
                            stop=True,
                            tile_position=(r0, 64 * sub),
                        )
                pv = probs[:, g, :]
                pv3 = probs[:, g, :].rearrange("p (r n) -> p r n", r=4)
                nc.scalar.activation(pv3, sc4[:, :, 0 : 2 * N], mybir.ActivationFunctionType.Exp)
                nc.vector.tensor_mul(pv, pv, eab_sb[:])
                k_sec, j = g // 16, g % 16
                nc.tensor.matmul(
                    sums[32 * k_sec : 32 * k_sec + 32, :],
                    mask_sb[:, j, :],
                    pv,
                    start=(j == 0),
                    stop=(j == 15),
                    tile_position=(0, 32 * k_sec),
                )

            # ---- P4: reciprocal of denominators ----
            rec = rc_p.tile([128, GW], F32, tag="recf")
            nc.vector.reciprocal(rec[:], sums[:])
            recb = rc_p.tile([128, GW], BF16, tag="recb")
            nc.vector.tensor_copy(recb[:], rec[:])
            recd = dr_p.tile([128, GW], BF16, tag="recd")
            nc.gpsimd.dma_start(recd[:], recb[:])

            # ---- P5: v projection, attention out, final projection ----
            v_tiles = [None] * NCHUNK
            hT_tiles = [None] * NCHUNK

            def emit_proj(cc):
                hTt = hT_tiles[cc]
                for t in range(7):
                    psy = sh_ps.tile([128, DIM], F32, tag="shps")
                    for h in range(HEADS):
                        nc.tensor.matmul(
                            psy[:],
                            hTt[:, h, 128 * t : 128 * (t + 1)],
                            pw_sb[:, h, :],
                            start=(h == 0),
                            stop=(h == HEADS - 1),
                        )
                    yt = y_p.tile([128, DIM], BF16, tag="yt")
                    nc.vector.tensor_copy(yt[:], psy[:])
                    g0 = ht0 + cc * CHUNK + t * 128
                    nc.gpsimd.dma_start(y_d[g0 : g0 + 128, :], yt[:])

            for c in range(NCHUNK):
                c0 = c * CHUNK
                vt = v_p.tile([128, 7, DH], BF16, tag="vch")
                v_tiles[c] = vt
                hTt = hT_p.tile([128, HEADS, CHUNK], BF16, tag="hT")
                hT_tiles[c] = hTt
                for t in range(7):
                    t0 = c0 + t * 128
                    pv1 = sh_ps.tile([128, 512], F32, tag="shps")
                    pv2 = sh_ps.tile([128, 512], F32, tag="shps")
                    for f in range(3):
                        nc.tensor.matmul(
                            pv1[:], xT[:, f, t0 : t0 + 128], vw_sb[:, f, 0:512],
                            start=(f == 0), stop=(f == 2),
                        )
                        nc.tensor.matmul(
                            pv2[:], xT[:, f, t0 : t0 + 128], vw_sb[:, f, 512:1024],
                            start=(f == 0), stop=(f == 2),
                        )
                    nc.vector.tensor_copy(vt[:, t, 0:512], pv1[:])
                    nc.vector.tensor_copy(vt[:, t, 512:1024], pv2[:])

                # groups whose last token falls in this chunk
                for g in range(64):
                    glast = 2 * N * g + 2 * N - 1
                    if glast // CHUNK != c:
                        continue
                    # normalize probs for this group
                    bt = bc_p.tile([128, GW], BF16, tag="bc")
                    for sub in range(2):
                        row = recd[2 * g + sub : 2 * g + sub + 1, :]
                        src = bass.AP(
                            tensor=row.tensor,
                            offset=row.offset,
                            ap=[[0, 64]] + list(row.ap[1:]),
                        )
                        nc.gpsimd.dma_start(bt[64 * sub : 64 * sub + 64, :], src)
                    nc.vector.tensor_mul(probs[:, g, :], probs[:, g, :], bt[:])
                    # re-layout v rows of both batches to partitions 0/64
                    v2 = v2_p.tile([128, DH], BF16, tag="v2")
                    for sub in range(2):
                        tok0 = N * (2 * g + sub)
                        i0, r0 = tok0 // 128, tok0 % 128
                        segs = [(i0, r0, 0, min(N, 128 - r0))]
                        if segs[0][3] < N:
                            segs.append((i0 + 1, 0, segs[0][3], N - segs[0][3]))
                        for ti, pr0, m0, ml in segs:
                            vsrc = v_tiles[ti // 7]
                            nc.gpsimd.dma_start(
                                v2[64 * sub + m0 : 64 * sub + m0 + ml, :],
                                vsrc[pr0 : pr0 + ml, ti % 7, :],
                            )
                    for sub in range(2):
                        b = 2 * g + sub
                        ht_b = ht_ps.tile([128, 512], F32, tag="htps")
                        ht = ht_b[:, :GW]
                        for h in range(HEADS):
                            jh = 2 * (h % 4) + h // 4
                            nc.tensor.matmul(
                                ht[:, N * h : N * h + N],
                                v2[64 * sub : 64 * sub + N, 128 * h : 128 * (h + 1)],
                                probs[64 * sub : 64 * sub + N, g, N * jh : N * jh + N],
                                start=True,
                                stop=True,
                                tile_position=(64 * sub, 0),
                            )
                        # evacuate ht -> hT chunk(s), splitting at chunk boundary
                        htv = ht[:].rearrange("p (h n) -> p h n", h=HEADS)
                        tok0 = N * b
                        cc0 = tok0 // CHUNK
                        segs = [(cc0, tok0 % CHUNK, 0, min(N, CHUNK * (cc0 + 1) - tok0))]
                        if segs[0][3] < N:
                            segs.append((cc0 + 1, 0, segs[0][3], N - segs[0][3]))
                        for scc, d0, s0, w in segs:
                            nc.vector.tensor_copy(
                                hT_tiles[scc][:, :, d0 : d0 + w], htv[:, :, s0 : s0 + w]
                            )
                if c > 0:
                    emit_proj(c - 1)
            emit_proj(NCHUNK - 1)

    nc.compile()
    return nc


def _host_prep(q_w, k_w, v_w, proj_w, attention_biases, bias_idxs):
    ab = np.asarray(attention_biases)[:, np.asarray(bias_idxs)]  # [H, N, N]
    eab = np.ones((128, GW), dtype=np.float32)
    e = np.exp(ab)  # [H, n, m]
    for h in range(HEADS):
        jh = 2 * (h % 4) + h // 4  # head h lives at column block jh
        ehT = e[h].T  # [m, n]
        eab[0:N, N * jh : N * jh + N] = ehT
        eab[64 : 64 + N, N * jh : N * jh + N] = ehT
    mask = np.zeros((16, 128, 32), dtype=np.float32)
    for j in range(16):
        mask[j, 0:N, 2 * j] = 1.0
        mask[j, 64 : 64 + N, 2 * j + 1] = 1.0
    consts = {
        "qw": np.ascontiguousarray(
            (np.asarray(q_w).T * SCALE).reshape(3, 128, NHKD).astype(NPBF)
        ),
        "kw": np.ascontiguousarray(np.asarray(k_w).T.reshape(3, 128, NHKD).astype(NPBF)),
        "vw": np.ascontiguousarray(np.asarray(v_w).T.reshape(3, 128, DH).astype(NPBF)),
        "pw": np.ascontiguousarray(
            np.asarray(proj_w).T.reshape(HEADS, 128, DIM).astype(NPBF)
        ),
        "eab": eab.astype(NPBF),
        "mask": mask.astype(NPBF),
        "ident": np.eye(128, dtype=np.float32).astype(NPBF),
    }
    return consts


class _Runner:
    """Caches the compiled jit(shard_map(bass_exec)) and device-resident
    consts; per call only x moves host->device and y device->host."""

    def __init__(self):
        import jax
        from jax.sharding import Mesh, PartitionSpec, NamedSharding
        from jax.experimental.shard_map import shard_map
        from concourse import bass2jax

        self.jax = jax
        bass2jax.install_neuronx_cc_hook()
        nc = _build()
        self.nc = nc

        part_name = (
            nc.partition_id_tensor.name if nc.partition_id_tensor else None
        )
        in_names, out_names, out_avals = [], [], []
        for alloc in nc.m.functions[0].allocations:
            if not isinstance(alloc, mybir.MemoryLocationSet):
                continue
            name = alloc.memorylocations[0].name
            if alloc.kind == "ExternalInput":
                if name != part_name:
                    in_names.append(name)
            elif alloc.kind == "ExternalOutput":
                out_names.append(name)
                out_avals.append(
                    jax.core.ShapedArray(
                        tuple(alloc.tensor_shape), mybir.dt.np(alloc.dtype)
                    )
                )
        assert out_names == ["y"], out_names
        self.in_names = list(in_names)
        all_names = list(in_names) + list(out_names)
        if part_name is not None:
            all_names.append(part_name)
        all_names = tuple(all_names)
        out_avals_t = tuple(out_avals)

        self.devices = jax.devices()[:NCORES]
        mesh = Mesh(np.asarray(self.devices), ("core",))
        self.mesh = mesh
        P = PartitionSpec
        self.sharding = NamedSharding(mesh, P("core"))

        dbg_name = nc.dbg_addr.name if nc.dbg_addr is not None else None

        def _body(*args):
            operands = list(args)
            if part_name is not None:
                operands.append(bass2jax.partition_id_tensor())
            outs = bass2jax._bass_exec_p.bind(
                *operands,
                out_avals=out_avals_t,
                in_names=all_names,
                out_names=tuple(out_names),
                lowering_input_output_aliases=(),
                sim_require_finite=True,
                sim_require_nnan=True,
                nc=nc,
            )
            return tuple(outs)

        nin = len(in_names) + len(out_names)
        self.fn = jax.jit(
            shard_map(
                _body,
                mesh=mesh,
                in_specs=(P("core"),) * nin,
                out_specs=(P("core"),) * len(out_names),
                check_rep=False,
            ),
            donate_argnums=(nin - 1,),
            keep_unused=True,
        )

        self._dbg_name = dbg_name
        self._consts_key = None
        self._consts_dev = None  # dict name -> device array (global, sharded)
        self._spare = None       # donation fodder for y (device array)
        self._pool = ThreadPoolExecutor(max_workers=NCORES)

    def _put_consts(self, consts):
        key = b"".join(consts[k].tobytes() for k in sorted(consts))
        if self._consts_key == key:
            return
        dev = {}
        for name, arr in consts.items():
            g = np.concatenate([arr] * NCORES, axis=0)
            dev[name] = self.jax.device_put(g, self.sharding)
        if self._dbg_name is not None:
            g = np.zeros((NCORES, 2), np.uint32)
            dev[self._dbg_name] = self.jax.device_put(g, self.sharding)
        self._consts_key = key
        self._consts_dev = dev

    def _put_x(self, x):
        # x: [B, N, DIM] f32 host. Global device layout: [B*N, DIM] bf16
        # sharded on rows. Cast + ship each core's shard in parallel.
        x2 = np.ascontiguousarray(x, dtype=np.float32).reshape(B * N, DIM)

        def one(i):
            sl = x2[i * T : (i + 1) * T]
            return self.jax.device_put(sl.astype(NPBF), self.devices[i])

        shards = list(self._pool.map(one, range(NCORES)))
        return self.jax.make_array_from_single_device_arrays(
            (B * N, DIM), self.sharding, shards
        )

    def _fetch_y(self, y_dev):
        out = np.empty((B * N, DIM), np.float32)

        def one(shard):
            i = shard.index[0].start // T if shard.index[0].start else 0
            buf = np.asarray(shard.data)  # bf16 [T, DIM]
            np.copyto(out[i * T : (i + 1) * T], buf)  # cast bf16 -> f32

        list(self._pool.map(one, y_dev.addressable_shards))
        return out.reshape(B, N, DIM)

    def __call__(self, inputs):
        consts = _host_prep(
            inputs["q_w"], inputs["k_w"], inputs["v_w"], inputs["proj_w"],
            inputs["attention_biases"], inputs["bias_idxs"],
        )
        self._put_consts(consts)
        x_dev = self._put_x(np.asarray(inputs["x"]))
        if self._spare is None:
            z = np.zeros((B * N, DIM), NPBF)
            self._spare = self.jax.device_put(z, self.sharding)
        args = [x_dev if n == "x" else self._consts_dev[n] for n in self.in_names]
        args.append(self._spare)
        self._spare = None
        (y_dev,) = self.fn(*args)
        y = self._fetch_y(y_dev)
        self._spare = y_dev  # reuse as next call's donated output scratch
        return y


_runner = None


def _legacy_trace_run(inputs, trace=True, **kw):
    """Traced path through run_bass_kernel_spmd for profiling (slow)."""
    from concourse.bass_utils import run_bass_kernel_spmd

    global _runner
    if _runner is None:
        _runner = _Runner()
    x = np.asarray(inputs["x"], dtype=np.float32)
    consts = _host_prep(
        inputs["q_w"], inputs["k_w"], inputs["v_w"], inputs["proj_w"],
        inputs["attention_biases"], inputs["bias_idxs"],
    )
    xs = x.reshape(NCORES, T, DIM)
    in_maps = [
        dict(
            {k: v for k, v in consts.items()},
            x=np.ascontiguousarray(xs[i]).astype(NPBF),
        )
        for i in range(NCORES)
    ]
    res = run_bass_kernel_spmd(
        _runner.nc, in_maps, core_ids=list(range(NCORES)), trace=trace, **kw
    )
    y = np.stack([r["y"] for r in res.results], axis=0).astype(np.float32)
    return y.reshape(B, N, DIM), res


class _NoTraceRes:
    exec_time_ns = None
    instructions_and_trace = None
    results = None


def run(inputs, trace=False, **kw):
    global _runner
    if trace:
        return _legacy_trace_run(inputs, trace=True, **kw)
    if _runner is None:
        _runner = _Runner()
    y = _runner(inputs)
    return y, _NoTraceRes()


def kernel(**inputs):
    y, _ = run(inputs)
    return y
